# revision 7
# baseline (speedup 1.0000x reference)
"""Mixtral-style MoE (top-2 of 8 experts) on 8 TRN2 NeuronCores.

Strategy (expert-parallel, matching TENSOR_EXPERT_PARALLEL):
  - Host: router (logits -> softmax -> top-2 -> normalized weights), then
    shard: core e receives up to C=1024 tokens routed to expert e
    (gathered and pre-transposed to [H, C]) plus expert e's w1/w3/w2
    (bf16, pre-packed into PE-friendly [128 x free] tiles).  C=1024 is
    chosen so every core does identical, full-partition work (8 chunks
    of 128 tokens); the few overflow pairs beyond 1024 per expert
    (~1.3% of the 8192 token-expert pairs at balanced routing) are
    computed exactly on the host in fp32 and added into the output.
  - Device (SPMD, identical program on 8 cores): h1T = w1 @ xeT,
    h3T = w3 @ xeT, gT = silu(h1T) * h3T (bf16), outT = gT.T @ w2T,
    scaled per-token by the routing weight.  Pure GEMM pipeline; all
    DMAs are fully linear.
  - Host: scatter-add each core's [count_e, H] contribution into the
    [T, H] output (each token appears in exactly TOP_K=2 expert lists).

Compute is done in bf16 (fp32 accumulation in PSUM), which keeps the
TensorEngine at its 78.6 TF/s peak; sparse routing means each core does
C = 1024 token-columns instead of all 4096 (4x fewer FLOPs than dense).
"""

import numpy as np
import ml_dtypes

B, S, H, F, E, TOP_K = 2, 2048, 1024, 3584, 8, 2
N_CORES = 8
P = 128
HK = H // P   # 8 contraction chunks for up-proj
FP = F // P   # 28 partition chunks of the FFN dim
C = 1024      # per-core token capacity (8 full 128-token chunks)
CK = C // P
NWARM = 4     # HAM warmup matmuls (PE clock-gate ramp)

BF16 = ml_dtypes.bfloat16

_BUILD_CACHE = {}
LAST_EXEC_TIME_NS = None


def _ensure_axon_hooks_stub():
    """bass_utils imports antenv.axon_hooks when BASS_TRACE is set; the
    agent image lacks it.  Register a None-hook stub so a stray
    BASS_TRACE env var degrades to an untraced run instead of crashing.
    """
    import sys, types

    try:
        import antenv.axon_hooks  # noqa: F401
        return
    except ImportError:
        pass
    mod = types.ModuleType("antenv.axon_hooks")
    mod._hook = None
    mod.set_axon_ntff_profile_hook = lambda h: setattr(mod, "_hook", h)
    mod.get_axon_ntff_profile_hook = lambda: mod._hook
    sys.modules["antenv.axon_hooks"] = mod
    try:
        import antenv

        antenv.axon_hooks = mod
    except ImportError:
        pass


def _build():
    """Build + compile the SPMD Bass program (token capacity C=1024)."""
    import concourse.bacc as bacc
    import concourse.mybir as mybir
    from concourse.tile import TileContext

    bf = mybir.dt.bfloat16
    f32 = mybir.dt.float32

    nc = bacc.Bacc("TRN2", target_bir_lowering=False, debug=False,
                   num_devices=N_CORES)
    xe = nc.dram_tensor("xe", [HK, P, C], bf, kind="ExternalInput")
    w1p = nc.dram_tensor("w1p", [FP, P, H], bf, kind="ExternalInput")
    w3p = nc.dram_tensor("w3p", [FP, P, H], bf, kind="ExternalInput")
    w2p = nc.dram_tensor("w2p", [FP, P, H], bf, kind="ExternalInput")
    cv = nc.dram_tensor("cv", [C, 1], f32, kind="ExternalInput")
    out = nc.dram_tensor("out", [C, H], bf, kind="ExternalOutput")

    cn_chunks = [(0, 512), (512, 512)]
    silu = mybir.ActivationFunctionType.Silu
    copy = mybir.ActivationFunctionType.Copy

    with TileContext(nc) as tc:
        with (
            tc.tile_pool(name="persist", bufs=1) as persist,
            tc.tile_pool(name="wload", bufs=3) as wload,
            tc.tile_pool(name="gpool", bufs=1) as gpool,
            tc.tile_pool(name="evac", bufs=4) as evac,
            tc.tile_pool(name="ost", bufs=3) as ost,
        ):
            # Startup-critical DMA order: fp0 panels, first activation
            # chunks, fp1 panels, rest of the activations.  fp0's hk-outer
            # matmul chain consumes xe[hk] at ~1us/chunk, so fp1's panels
            # slot in without delaying it.  DIRECT2D issue costs ~0.6us
            # each on a sequencer, so the critical set is issued first,
            # split across the two HWDGE engines (Sync + Scalar).
            w1t0 = wload.tile([P, H], bf, tag="w1")
            nc.sync.dma_start(out=w1t0[:], in_=w1p[0])
            w3t0 = wload.tile([P, H], bf, tag="w3")
            nc.scalar.dma_start(out=w3t0[:], in_=w3p[0])
            w1t1 = wload.tile([P, H], bf, tag="w1")
            nc.sync.dma_start(out=w1t1[:], in_=w1p[1])
            w3t1 = wload.tile([P, H], bf, tag="w3")
            nc.scalar.dma_start(out=w3t1[:], in_=w3p[1])

            xet = [persist.tile([P, C], bf, tag=f"xe{hk}", name=f"xe{hk}")
                   for hk in range(HK)]
            # xe[0] is needed first: split it across both engines so its
            # completion semaphores fire ~1us sooner.
            nc.sync.dma_start(out=xet[0][:, 0:512], in_=xe[0][:, 0:512])
            nc.scalar.dma_start(out=xet[0][:, 512:C], in_=xe[0][:, 512:C])
            for hk in range(1, HK):
                e = nc.sync if hk % 2 == 0 else nc.scalar
                e.dma_start(out=xet[hk][:], in_=xe[hk])

            gt = [gpool.tile([P, C], bf, tag=f"g{fp}", name=f"g{fp}")
                  for fp in range(FP)]

            # HAM warmup: the PE clock-gate needs ~3.4us of sustained
            # activity to lift 1.2 -> 2.4 GHz.  Burn the initial DMA wait
            # on dummy matmuls so real matmuls start near full rate.  The
            # operand memset runs on the otherwise-idle GpSimd engine so
            # nothing upstream delays the dummies; result is discarded.
            # The warm tile comes FIRST from the shared psA pool so the
            # first real matmul chain lands on a different PSUM bank and
            # never waits on the warmup's completion.
            warm = persist.tile([P, 512], bf, tag="warm", name="warm")
            nc.gpsimd.memset(warm[:], 0.0)

            # Phase A: h1T/h3T = w1/w3 @ xeT per 128-row chunk of F,
            # fused SwiGLU into gT (bf16).
            with tc.tile_pool(name="psA", bufs=4, space="PSUM") as psA:
                # Tagged ps1 so it occupies one rotation slot of that tag;
                # the first real ps1 chain gets the next slot and starts
                # without waiting on the warmup matmuls.
                wps = psA.tile([P, 512], f32, tag="ps1", name="wps")
                for i in range(NWARM):
                    nc.tensor.matmul(wps[:], warm[:, 0:P], warm[:],
                                     start=True, stop=True)
                # fp0 + fp1 in hk-LOCKSTEP: 8 matmuls consume each xe[hk]
                # chunk as it lands (~1.7us of PE per 256KB chunk), so the
                # startup DMA stream keeps ahead of the PE with margin.
                pss = {}
                for fp in (0, 1):
                    for mat in (1, 3):
                        for ci in range(len(cn_chunks)):
                            pss[(fp, mat, ci)] = psA.tile(
                                [P, 512], f32, tag=f"ps{mat}",
                                name=f"ps{mat}_f{fp}_c{ci}",
                            )
                wts = {(0, 1): w1t0, (0, 3): w3t0, (1, 1): w1t1, (1, 3): w3t1}
                for hk in range(HK):
                    for fp in (0, 1):
                        for mat in (1, 3):
                            wt = wts[(fp, mat)]
                            for ci, (coff, csz) in enumerate(cn_chunks):
                                nc.tensor.matmul(
                                    pss[(fp, mat, ci)][:, :csz],
                                    wt[:, hk * P:(hk + 1) * P],
                                    xet[hk][:, coff:coff + csz],
                                    start=(hk == 0), stop=(hk == HK - 1),
                                )
                for fp in (0, 1):
                    for ci, (coff, csz) in enumerate(cn_chunks):
                        sil = evac.tile([P, 512], f32, tag="sil",
                                        name=f"sil_f{fp}_{ci}")
                        nc.scalar.activation(
                            sil[:, :csz], pss[(fp, 1, ci)][:, :csz], silu)
                        nc.vector.tensor_mul(
                            gt[fp][:, coff:coff + csz], sil[:, :csz],
                            pss[(fp, 3, ci)][:, :csz],
                        )
                for fp in range(2, FP):
                    w1t = wload.tile([P, H], bf, tag="w1")
                    nc.sync.dma_start(out=w1t[:], in_=w1p[fp])
                    w3t = wload.tile([P, H], bf, tag="w3")
                    nc.sync.dma_start(out=w3t[:], in_=w3p[fp])
                    for (coff, csz) in cn_chunks:
                        ps1 = psA.tile([P, 512], f32, tag="ps1")
                        ps3 = psA.tile([P, 512], f32, tag="ps3")
                        for hk in range(HK):
                            nc.tensor.matmul(
                                ps1[:, :csz],
                                w1t[:, hk * P:(hk + 1) * P],
                                xet[hk][:, coff:coff + csz],
                                start=(hk == 0), stop=(hk == HK - 1),
                            )
                        for hk in range(HK):
                            nc.tensor.matmul(
                                ps3[:, :csz],
                                w3t[:, hk * P:(hk + 1) * P],
                                xet[hk][:, coff:coff + csz],
                                start=(hk == 0), stop=(hk == HK - 1),
                            )
                        sil = evac.tile([P, 512], f32, tag="sil")
                        nc.scalar.activation(sil[:, :csz], ps1[:, :csz], silu)
                        nc.vector.tensor_mul(
                            gt[fp][:, coff:coff + csz], sil[:, :csz], ps3[:, :csz]
                        )

            # w2 / routing-weight loads are only needed in phase B; emit
            # them after phase A so the DMA queues serve phase A first.
            cvt = []
            for ck in range(CK):
                t = persist.tile([P, 1], f32, tag=f"cv{ck}", name=f"cv{ck}")
                nc.sync.dma_start(out=t[:], in_=cv[ck * P:(ck + 1) * P, :])
                cvt.append(t)
            w2t = []
            for fp in range(FP):
                t = persist.tile([P, H], bf, tag=f"w2_{fp}", name=f"w2_{fp}")
                nc.sync.dma_start(out=t[:], in_=w2p[fp])
                w2t.append(t)

            # Phase B: outT chunk [128 tokens, 1024] = sum_f gT.T @ w2T,
            # scaled by the per-token routing weight on eviction.  The two
            # 512-column halves run as back-to-back full accumulation
            # chains; each half is evicted (bf16) and DMA'd as soon as its
            # chain completes — pb0 on Scalar+Sync, pb1 on Vector+Scalar —
            # so the end-of-kernel drain is one half-eviction deep.
            with tc.tile_pool(name="psB", bufs=4, space="PSUM") as psB:
                for ck in range(CK):
                    pb0 = psB.tile([P, 512], f32, tag="pb0")
                    pb1 = psB.tile([P, 512], f32, tag="pb1")
                    for fp in range(FP):
                        nc.tensor.matmul(pb0[:], gt[fp][:, ck * P:(ck + 1) * P],
                                         w2t[fp][:, 0:512],
                                         start=(fp == 0), stop=(fp == FP - 1))
                    o0 = ost.tile([P, 512], bf, tag="o0")
                    nc.scalar.activation(o0[:], pb0[:], copy,
                                         scale=cvt[ck][:])
                    nc.sync.dma_start(out=out[ck * P:(ck + 1) * P, 0:512],
                                      in_=o0[:])
                    for fp in range(FP):
                        nc.tensor.matmul(pb1[:], gt[fp][:, ck * P:(ck + 1) * P],
                                         w2t[fp][:, 512:1024],
                                         start=(fp == 0), stop=(fp == FP - 1))
                    o1 = ost.tile([P, 512], bf, tag="o1")
                    nc.vector.tensor_scalar_mul(o1[:], pb1[:], cvt[ck][:])
                    nc.scalar.dma_start(out=out[ck * P:(ck + 1) * P, 512:1024],
                                        in_=o1[:])

    nc.compile()
    return nc


def _silu(v):
    return v / (1.0 + np.exp(-v))


def kernel(hidden_states, gate_w, w1, w2, w3, _trace=False):
    global LAST_EXEC_TIME_NS
    _ensure_axon_hooks_stub()
    from concourse.bass_utils import run_bass_kernel_spmd

    x = np.asarray(hidden_states, dtype=np.float32).reshape(-1, H)
    gate_w = np.asarray(gate_w, dtype=np.float32)
    w1 = np.asarray(w1, dtype=np.float32)
    w2 = np.asarray(w2, dtype=np.float32)
    w3 = np.asarray(w3, dtype=np.float32)
    T = x.shape[0]

    # Router (f32, same math as the module): softmax over experts, top-2,
    # renormalized weights.
    logits = x @ gate_w.T
    p = np.exp(logits - logits.max(-1, keepdims=True))
    p /= p.sum(-1, keepdims=True)
    sel = np.argpartition(-p, TOP_K - 1, axis=-1)[:, :TOP_K]
    rw = np.take_along_axis(p, sel, axis=-1)
    rw = rw / rw.sum(-1, keepdims=True)

    idx_e, cv_e = [], []
    for e in range(E):
        hit = sel == e                      # [T, K]
        idx = np.nonzero(hit.any(axis=1))[0]
        w = np.where(hit[idx, 0], rw[idx, 0], rw[idx, 1])
        idx_e.append(idx)
        cv_e.append(w.astype(np.float32))

    if "nc" not in _BUILD_CACHE:
        _BUILD_CACHE["nc"] = _build()
    nc = _BUILD_CACHE["nc"]

    x_bf = x.astype(BF16)
    in_maps = []
    for e in range(E):
        idx = idx_e[e][:C]
        n = len(idx)
        xeT = np.zeros((H, C), dtype=BF16)
        xeT[:, :n] = x_bf[idx].T
        cvp = np.zeros((C, 1), dtype=np.float32)
        cvp[:n, 0] = cv_e[e][:n]
        w1pk = np.ascontiguousarray(
            w1[e].astype(BF16).reshape(FP, P, HK, P).transpose(0, 3, 2, 1)
        ).reshape(FP, P, H)
        w3pk = np.ascontiguousarray(
            w3[e].astype(BF16).reshape(FP, P, HK, P).transpose(0, 3, 2, 1)
        ).reshape(FP, P, H)
        w2pk = np.ascontiguousarray(w2[e].T.astype(BF16)).reshape(FP, P, H)
        in_maps.append({
            "xe": np.ascontiguousarray(xeT.reshape(HK, P, C)),
            "w1p": w1pk,
            "w3p": w3pk,
            "w2p": w2pk,
            "cv": cvp,
        })

    res = run_bass_kernel_spmd(
        nc, in_maps, core_ids=list(range(N_CORES)), trace=_trace
    )
    LAST_EXEC_TIME_NS = res.exec_time_ns

    out = np.zeros((T, H), dtype=np.float32)
    for e in range(E):
        idx = idx_e[e][:C]
        n = len(idx)
        if n:
            dev = np.asarray(res.results[e]["out"], dtype=np.float32)
            out[idx] += dev.reshape(C, H)[:n]
        # Capacity overflow (tokens beyond C for this expert): exact
        # host-side fp32 patch.  ~1.3% of pairs at balanced routing.
        ov = idx_e[e][C:]
        if len(ov):
            X = x[ov]
            h1 = X @ w1[e].T
            h3 = X @ w3[e].T
            g = _silu(h1) * h3 * cv_e[e][C:, None]
            out[ov] += g @ w2[e].T
    return out.reshape(B, S, H)


# revision 8
# speedup vs baseline: 1.1899x; 1.1899x over previous
"""Mixtral-style MoE (top-2 of 8 experts) on 8 TRN2 NeuronCores.

Strategy (expert-parallel, matching TENSOR_EXPERT_PARALLEL):
  - Host: router (logits -> softmax -> top-2 -> normalized weights), then
    shard: core e receives up to C=1024 tokens routed to expert e
    (gathered and pre-transposed to [H, C]) plus expert e's w1/w3/w2
    (bf16, pre-packed into PE-friendly [128 x free] tiles).  C=1024 is
    chosen so every core does identical, full-partition work (8 chunks
    of 128 tokens); the few overflow pairs beyond 1024 per expert
    (~1.3% of the 8192 token-expert pairs at balanced routing) are
    computed exactly on the host in fp32 and added into the output.
  - Device (SPMD, identical program on 8 cores): h1T = w1 @ xeT,
    h3T = w3 @ xeT, gT = silu(h1T) * h3T (bf16), outT = gT.T @ w2T,
    scaled per-token by the routing weight.  Pure GEMM pipeline; all
    DMAs are fully linear.
  - Host: scatter-add each core's [count_e, H] contribution into the
    [T, H] output (each token appears in exactly TOP_K=2 expert lists).

Compute is done in bf16 (fp32 accumulation in PSUM), which keeps the
TensorEngine at its 78.6 TF/s peak; sparse routing means each core does
C = 1024 token-columns instead of all 4096 (4x fewer FLOPs than dense).
"""

import numpy as np
import ml_dtypes

B, S, H, F, E, TOP_K = 2, 2048, 1024, 3584, 8, 2
N_CORES = 8
P = 128
HK = H // P   # 8 contraction chunks for up-proj
FP = F // P   # 28 partition chunks of the FFN dim
C = 1024      # per-core token capacity (8 full 128-token chunks)
CK = C // P
NWARM = 4     # HAM warmup matmuls (PE clock-gate ramp)

BF16 = ml_dtypes.bfloat16

_BUILD_CACHE = {}
LAST_EXEC_TIME_NS = None


def _ensure_axon_hooks_stub():
    """bass_utils imports antenv.axon_hooks when BASS_TRACE is set; the
    agent image lacks it.  Register a None-hook stub so a stray
    BASS_TRACE env var degrades to an untraced run instead of crashing.
    """
    import sys, types

    try:
        import antenv.axon_hooks  # noqa: F401
        return
    except ImportError:
        pass
    mod = types.ModuleType("antenv.axon_hooks")
    mod._hook = None
    mod.set_axon_ntff_profile_hook = lambda h: setattr(mod, "_hook", h)
    mod.get_axon_ntff_profile_hook = lambda: mod._hook
    sys.modules["antenv.axon_hooks"] = mod
    try:
        import antenv

        antenv.axon_hooks = mod
    except ImportError:
        pass


def _build():
    """Build + compile the SPMD Bass program (token capacity C=1024)."""
    import concourse.bacc as bacc
    import concourse.mybir as mybir
    from concourse.tile import TileContext

    bf = mybir.dt.bfloat16
    f32 = mybir.dt.float32

    nc = bacc.Bacc("TRN2", target_bir_lowering=False, debug=False,
                   num_devices=N_CORES)
    xe = nc.dram_tensor("xe", [HK, P, C], bf, kind="ExternalInput")
    w1p = nc.dram_tensor("w1p", [FP, P, H], bf, kind="ExternalInput")
    w3p = nc.dram_tensor("w3p", [FP, P, H], bf, kind="ExternalInput")
    w2p = nc.dram_tensor("w2p", [FP, P, H], bf, kind="ExternalInput")
    cv = nc.dram_tensor("cv", [C, 1], f32, kind="ExternalInput")
    out = nc.dram_tensor("out", [C, H], bf, kind="ExternalOutput")

    cn_chunks = [(0, 512), (512, 512)]
    silu = mybir.ActivationFunctionType.Silu
    copy = mybir.ActivationFunctionType.Copy

    with TileContext(nc) as tc:
        with (
            tc.tile_pool(name="persist", bufs=1) as persist,
            tc.tile_pool(name="wload", bufs=3) as wload,
            tc.tile_pool(name="gpool", bufs=1) as gpool,
            tc.tile_pool(name="evac", bufs=4) as evac,
            tc.tile_pool(name="ost", bufs=3) as ost,
        ):
            # Startup-critical DMA order: fp0 panels, first activation
            # chunks, fp1 panels, rest of the activations.  fp0's hk-outer
            # matmul chain consumes xe[hk] at ~1us/chunk, so fp1's panels
            # slot in without delaying it.  DIRECT2D issue costs ~0.6us
            # each on a sequencer, so the critical set is issued first,
            # split across the two HWDGE engines (Sync + Scalar).
            # The fp0/fp1 panels are split into the hk0 chunk (32KB,
            # needed by the very first matmul group) and the rest, so the
            # critical first-group set (4x32KB + xe[0]) clears the DMA
            # queues in ~2.7us instead of draining 1MB of full panels.
            w1t0 = wload.tile([P, H], bf, tag="w1")
            w3t0 = wload.tile([P, H], bf, tag="w3")
            w1t1 = wload.tile([P, H], bf, tag="w1")
            w3t1 = wload.tile([P, H], bf, tag="w3")
            nc.sync.dma_start(out=w1t0[:, 0:P], in_=w1p[0][:, 0:P])
            nc.scalar.dma_start(out=w3t0[:, 0:P], in_=w3p[0][:, 0:P])
            nc.sync.dma_start(out=w1t1[:, 0:P], in_=w1p[1][:, 0:P])
            nc.scalar.dma_start(out=w3t1[:, 0:P], in_=w3p[1][:, 0:P])

            xet = [persist.tile([P, C], bf, tag=f"xe{hk}", name=f"xe{hk}")
                   for hk in range(HK)]
            # xe[0] next: split across both engines so its completion
            # semaphores fire ~1us sooner.
            nc.sync.dma_start(out=xet[0][:, 0:512], in_=xe[0][:, 0:512])
            nc.scalar.dma_start(out=xet[0][:, 512:C], in_=xe[0][:, 512:C])
            # Panel remainders (needed from the second matmul group on),
            # interleaved with the remaining activation chunks in
            # consumption order.
            nc.sync.dma_start(out=w1t0[:, P:H], in_=w1p[0][:, P:H])
            nc.scalar.dma_start(out=w3t0[:, P:H], in_=w3p[0][:, P:H])
            nc.scalar.dma_start(out=xet[1][:], in_=xe[1])
            nc.sync.dma_start(out=w1t1[:, P:H], in_=w1p[1][:, P:H])
            nc.scalar.dma_start(out=w3t1[:, P:H], in_=w3p[1][:, P:H])
            for hk in range(2, HK):
                e = nc.sync if hk % 2 == 0 else nc.scalar
                e.dma_start(out=xet[hk][:], in_=xe[hk])

            gt = [gpool.tile([P, C], bf, tag=f"g{fp}", name=f"g{fp}")
                  for fp in range(FP)]

            # HAM warmup: the PE clock-gate needs ~3.4us of sustained
            # activity to lift 1.2 -> 2.4 GHz.  Burn the initial DMA wait
            # on dummy matmuls so real matmuls start near full rate.  The
            # operand memset runs on the otherwise-idle GpSimd engine so
            # nothing upstream delays the dummies; result is discarded.
            # The warm tile comes FIRST from the shared psA pool so the
            # first real matmul chain lands on a different PSUM bank and
            # never waits on the warmup's completion.
            warm = persist.tile([P, 512], bf, tag="warm", name="warm")
            nc.gpsimd.memset(warm[:], 0.0)

            # Phase A: h1T/h3T = w1/w3 @ xeT per 128-row chunk of F,
            # fused SwiGLU into gT (bf16).
            with tc.tile_pool(name="psA", bufs=4, space="PSUM") as psA:
                # Tagged ps1 so it occupies one rotation slot of that tag;
                # the first real ps1 chain gets the next slot and starts
                # without waiting on the warmup matmuls.
                wps = psA.tile([P, 512], f32, tag="ps1", name="wps")
                for i in range(NWARM):
                    nc.tensor.matmul(wps[:], warm[:, 0:P], warm[:],
                                     start=True, stop=True)
                # fp0 + fp1 in hk-LOCKSTEP: 8 matmuls consume each xe[hk]
                # chunk as it lands (~1.7us of PE per 256KB chunk), so the
                # startup DMA stream keeps ahead of the PE with margin.
                pss = {}
                for fp in (0, 1):
                    for mat in (1, 3):
                        for ci in range(len(cn_chunks)):
                            pss[(fp, mat, ci)] = psA.tile(
                                [P, 512], f32, tag=f"ps{mat}",
                                name=f"ps{mat}_f{fp}_c{ci}",
                            )
                wts = {(0, 1): w1t0, (0, 3): w3t0, (1, 1): w1t1, (1, 3): w3t1}
                for hk in range(HK):
                    for fp in (0, 1):
                        for mat in (1, 3):
                            wt = wts[(fp, mat)]
                            for ci, (coff, csz) in enumerate(cn_chunks):
                                nc.tensor.matmul(
                                    pss[(fp, mat, ci)][:, :csz],
                                    wt[:, hk * P:(hk + 1) * P],
                                    xet[hk][:, coff:coff + csz],
                                    start=(hk == 0), stop=(hk == HK - 1),
                                )
                for fp in (0, 1):
                    for ci, (coff, csz) in enumerate(cn_chunks):
                        sil = evac.tile([P, 512], f32, tag="sil",
                                        name=f"sil_f{fp}_{ci}")
                        nc.scalar.activation(
                            sil[:, :csz], pss[(fp, 1, ci)][:, :csz], silu)
                        nc.vector.tensor_mul(
                            gt[fp][:, coff:coff + csz], sil[:, :csz],
                            pss[(fp, 3, ci)][:, :csz],
                        )
                for fp in range(2, FP):
                    w1t = wload.tile([P, H], bf, tag="w1")
                    nc.sync.dma_start(out=w1t[:], in_=w1p[fp])
                    w3t = wload.tile([P, H], bf, tag="w3")
                    nc.sync.dma_start(out=w3t[:], in_=w3p[fp])
                    for (coff, csz) in cn_chunks:
                        ps1 = psA.tile([P, 512], f32, tag="ps1")
                        ps3 = psA.tile([P, 512], f32, tag="ps3")
                        for hk in range(HK):
                            nc.tensor.matmul(
                                ps1[:, :csz],
                                w1t[:, hk * P:(hk + 1) * P],
                                xet[hk][:, coff:coff + csz],
                                start=(hk == 0), stop=(hk == HK - 1),
                            )
                        for hk in range(HK):
                            nc.tensor.matmul(
                                ps3[:, :csz],
                                w3t[:, hk * P:(hk + 1) * P],
                                xet[hk][:, coff:coff + csz],
                                start=(hk == 0), stop=(hk == HK - 1),
                            )
                        sil = evac.tile([P, 512], f32, tag="sil")
                        nc.scalar.activation(sil[:, :csz], ps1[:, :csz], silu)
                        nc.vector.tensor_mul(
                            gt[fp][:, coff:coff + csz], sil[:, :csz], ps3[:, :csz]
                        )

            # w2 / routing-weight loads are only needed in phase B; emit
            # them after phase A so the DMA queues serve phase A first.
            cvt = []
            for ck in range(CK):
                t = persist.tile([P, 1], f32, tag=f"cv{ck}", name=f"cv{ck}")
                nc.sync.dma_start(out=t[:], in_=cv[ck * P:(ck + 1) * P, :])
                cvt.append(t)
            w2t = []
            for fp in range(FP):
                t = persist.tile([P, H], bf, tag=f"w2_{fp}", name=f"w2_{fp}")
                nc.sync.dma_start(out=t[:], in_=w2p[fp])
                w2t.append(t)

            # Phase B: outT chunk [128 tokens, 1024] = sum_f gT.T @ w2T,
            # scaled by the per-token routing weight on eviction.  The two
            # 512-column halves run as back-to-back full accumulation
            # chains; each half is evicted (bf16) and DMA'd as soon as its
            # chain completes — pb0 on Scalar+Sync, pb1 on Vector+Scalar —
            # so the end-of-kernel drain is one half-eviction deep.
            with tc.tile_pool(name="psB", bufs=4, space="PSUM") as psB:
                for ck in range(CK):
                    pb0 = psB.tile([P, 512], f32, tag="pb0")
                    pb1 = psB.tile([P, 512], f32, tag="pb1")
                    for fp in range(FP):
                        nc.tensor.matmul(pb0[:], gt[fp][:, ck * P:(ck + 1) * P],
                                         w2t[fp][:, 0:512],
                                         start=(fp == 0), stop=(fp == FP - 1))
                    o0 = ost.tile([P, 512], bf, tag="o0")
                    nc.scalar.activation(o0[:], pb0[:], copy,
                                         scale=cvt[ck][:])
                    nc.sync.dma_start(out=out[ck * P:(ck + 1) * P, 0:512],
                                      in_=o0[:])
                    for fp in range(FP):
                        nc.tensor.matmul(pb1[:], gt[fp][:, ck * P:(ck + 1) * P],
                                         w2t[fp][:, 512:1024],
                                         start=(fp == 0), stop=(fp == FP - 1))
                    o1 = ost.tile([P, 512], bf, tag="o1")
                    nc.vector.tensor_scalar_mul(o1[:], pb1[:], cvt[ck][:])
                    nc.scalar.dma_start(out=out[ck * P:(ck + 1) * P, 512:1024],
                                        in_=o1[:])

    nc.compile()
    return nc


def _silu(v):
    return v / (1.0 + np.exp(-v))


def kernel(hidden_states, gate_w, w1, w2, w3, _trace=False):
    global LAST_EXEC_TIME_NS
    _ensure_axon_hooks_stub()
    from concourse.bass_utils import run_bass_kernel_spmd

    x = np.asarray(hidden_states, dtype=np.float32).reshape(-1, H)
    gate_w = np.asarray(gate_w, dtype=np.float32)
    w1 = np.asarray(w1, dtype=np.float32)
    w2 = np.asarray(w2, dtype=np.float32)
    w3 = np.asarray(w3, dtype=np.float32)
    T = x.shape[0]

    # Router (f32, same math as the module): softmax over experts, top-2,
    # renormalized weights.
    logits = x @ gate_w.T
    p = np.exp(logits - logits.max(-1, keepdims=True))
    p /= p.sum(-1, keepdims=True)
    sel = np.argpartition(-p, TOP_K - 1, axis=-1)[:, :TOP_K]
    rw = np.take_along_axis(p, sel, axis=-1)
    rw = rw / rw.sum(-1, keepdims=True)

    idx_e, cv_e = [], []
    for e in range(E):
        hit = sel == e                      # [T, K]
        idx = np.nonzero(hit.any(axis=1))[0]
        w = np.where(hit[idx, 0], rw[idx, 0], rw[idx, 1])
        idx_e.append(idx)
        cv_e.append(w.astype(np.float32))

    if "nc" not in _BUILD_CACHE:
        _BUILD_CACHE["nc"] = _build()
    nc = _BUILD_CACHE["nc"]

    x_bf = x.astype(BF16)
    in_maps = []
    for e in range(E):
        idx = idx_e[e][:C]
        n = len(idx)
        xeT = np.zeros((H, C), dtype=BF16)
        xeT[:, :n] = x_bf[idx].T
        cvp = np.zeros((C, 1), dtype=np.float32)
        cvp[:n, 0] = cv_e[e][:n]
        w1pk = np.ascontiguousarray(
            w1[e].astype(BF16).reshape(FP, P, HK, P).transpose(0, 3, 2, 1)
        ).reshape(FP, P, H)
        w3pk = np.ascontiguousarray(
            w3[e].astype(BF16).reshape(FP, P, HK, P).transpose(0, 3, 2, 1)
        ).reshape(FP, P, H)
        w2pk = np.ascontiguousarray(w2[e].T.astype(BF16)).reshape(FP, P, H)
        in_maps.append({
            "xe": np.ascontiguousarray(xeT.reshape(HK, P, C)),
            "w1p": w1pk,
            "w3p": w3pk,
            "w2p": w2pk,
            "cv": cvp,
        })

    res = run_bass_kernel_spmd(
        nc, in_maps, core_ids=list(range(N_CORES)), trace=_trace
    )
    LAST_EXEC_TIME_NS = res.exec_time_ns

    out = np.zeros((T, H), dtype=np.float32)
    for e in range(E):
        idx = idx_e[e][:C]
        n = len(idx)
        if n:
            dev = np.asarray(res.results[e]["out"], dtype=np.float32)
            out[idx] += dev.reshape(C, H)[:n]
        # Capacity overflow (tokens beyond C for this expert): exact
        # host-side fp32 patch.  ~1.3% of pairs at balanced routing.
        ov = idx_e[e][C:]
        if len(ov):
            X = x[ov]
            h1 = X @ w1[e].T
            h3 = X @ w3[e].T
            g = _silu(h1) * h3 * cv_e[e][C:, None]
            out[ov] += g @ w2[e].T
    return out.reshape(B, S, H)


# revision 10
# speedup vs baseline: 1.1903x; 1.0003x over previous
"""Mixtral-style MoE (top-2 of 8 experts) on 8 TRN2 NeuronCores.

Strategy (expert-parallel, matching TENSOR_EXPERT_PARALLEL):
  - Host: router (logits -> softmax -> top-2 -> normalized weights), then
    shard: core e receives up to C=1024 tokens routed to expert e
    (gathered and pre-transposed to [H, C]) plus expert e's w1/w3/w2
    (bf16, pre-packed into PE-friendly [128 x free] tiles).  C=1024 is
    chosen so every core does identical, full-partition work (8 chunks
    of 128 tokens); the few overflow pairs beyond 1024 per expert
    (~1.3% of the 8192 token-expert pairs at balanced routing) are
    computed exactly on the host in fp32 and added into the output.
  - Device (SPMD, identical program on 8 cores): h1T = w1 @ xeT,
    h3T = w3 @ xeT, gT = silu(h1T) * h3T (bf16), outT = gT.T @ w2T,
    scaled per-token by the routing weight.  Pure GEMM pipeline; all
    DMAs are fully linear.
  - Host: scatter-add each core's [count_e, H] contribution into the
    [T, H] output (each token appears in exactly TOP_K=2 expert lists).

Compute is done in bf16 (fp32 accumulation in PSUM), which keeps the
TensorEngine at its 78.6 TF/s peak; sparse routing means each core does
C = 1024 token-columns instead of all 4096 (4x fewer FLOPs than dense).
"""

import numpy as np
import ml_dtypes

B, S, H, F, E, TOP_K = 2, 2048, 1024, 3584, 8, 2
N_CORES = 8
P = 128
HK = H // P   # 8 contraction chunks for up-proj
FP = F // P   # 28 partition chunks of the FFN dim
C = 1024      # per-core token capacity (8 full 128-token chunks)
CK = C // P
NWARM = 4     # HAM warmup matmuls (PE clock-gate ramp)

BF16 = ml_dtypes.bfloat16

_BUILD_CACHE = {}
LAST_EXEC_TIME_NS = None


def _ensure_axon_hooks_stub():
    """bass_utils imports antenv.axon_hooks when BASS_TRACE is set; the
    agent image lacks it.  Register a None-hook stub so a stray
    BASS_TRACE env var degrades to an untraced run instead of crashing.
    """
    import sys, types

    try:
        import antenv.axon_hooks  # noqa: F401
        return
    except ImportError:
        pass
    mod = types.ModuleType("antenv.axon_hooks")
    mod._hook = None
    mod.set_axon_ntff_profile_hook = lambda h: setattr(mod, "_hook", h)
    mod.get_axon_ntff_profile_hook = lambda: mod._hook
    sys.modules["antenv.axon_hooks"] = mod
    try:
        import antenv

        antenv.axon_hooks = mod
    except ImportError:
        pass


def _build():
    """Build + compile the SPMD Bass program (token capacity C=1024)."""
    import concourse.bacc as bacc
    import concourse.mybir as mybir
    from concourse.tile import TileContext

    bf = mybir.dt.bfloat16
    f32 = mybir.dt.float32

    nc = bacc.Bacc("TRN2", target_bir_lowering=False, debug=False,
                   num_devices=N_CORES)
    xe = nc.dram_tensor("xe", [HK, P, C], bf, kind="ExternalInput")
    w1p = nc.dram_tensor("w1p", [FP, P, H], bf, kind="ExternalInput")
    w3p = nc.dram_tensor("w3p", [FP, P, H], bf, kind="ExternalInput")
    w2p = nc.dram_tensor("w2p", [FP, P, H], bf, kind="ExternalInput")
    cv = nc.dram_tensor("cv", [C, 1], f32, kind="ExternalInput")
    out = nc.dram_tensor("out", [C, H], bf, kind="ExternalOutput")

    cn_chunks = [(0, 512), (512, 512)]
    silu = mybir.ActivationFunctionType.Silu
    copy = mybir.ActivationFunctionType.Copy

    with TileContext(nc) as tc:
        with (
            tc.tile_pool(name="persist", bufs=1) as persist,
            tc.tile_pool(name="wload", bufs=3) as wload,
            tc.tile_pool(name="gpool", bufs=1) as gpool,
            tc.tile_pool(name="evac", bufs=4) as evac,
            tc.tile_pool(name="ost", bufs=3) as ost,
        ):
            # Startup-critical DMA order: fp0 panels, first activation
            # chunks, fp1 panels, rest of the activations.  fp0's hk-outer
            # matmul chain consumes xe[hk] at ~1us/chunk, so fp1's panels
            # slot in without delaying it.  DIRECT2D issue costs ~0.6us
            # each on a sequencer, so the critical set is issued first,
            # split across the two HWDGE engines (Sync + Scalar).
            # The fp0/fp1 panels are split into the hk0 chunk (32KB,
            # needed by the very first matmul group) and the rest, so the
            # critical first-group set (4x32KB + xe[0]) clears the DMA
            # queues in ~2.7us instead of draining 1MB of full panels.
            w1t0 = wload.tile([P, H], bf, tag="w1")
            w3t0 = wload.tile([P, H], bf, tag="w3")
            w1t1 = wload.tile([P, H], bf, tag="w1")
            w3t1 = wload.tile([P, H], bf, tag="w3")
            nc.sync.dma_start(out=w1t0[:, 0:P], in_=w1p[0][:, 0:P])
            nc.scalar.dma_start(out=w3t0[:, 0:P], in_=w3p[0][:, 0:P])
            nc.sync.dma_start(out=w1t1[:, 0:P], in_=w1p[1][:, 0:P])
            nc.scalar.dma_start(out=w3t1[:, 0:P], in_=w3p[1][:, 0:P])

            xet = [persist.tile([P, C], bf, tag=f"xe{hk}", name=f"xe{hk}")
                   for hk in range(HK)]
            # xe[0] next: split across both engines so its completion
            # semaphores fire ~1us sooner.
            nc.sync.dma_start(out=xet[0][:, 0:512], in_=xe[0][:, 0:512])
            nc.scalar.dma_start(out=xet[0][:, 512:C], in_=xe[0][:, 512:C])

            # HAM warmup operand: memset first on the GpSimd queue (so
            # the dummy matmuls can start ~7us), then use GpSimd's SWDGE
            # as a THIRD parallel DMA-issue stream for xe[1..3] — the
            # chunks the lockstep needs while the two HWDGE queues are
            # still draining the weight-panel remainders.
            warm = persist.tile([P, 512], bf, tag="warm", name="warm")
            nc.gpsimd.memset(warm[:], 0.0)
            nc.gpsimd.dma_start(out=xet[1][:], in_=xe[1])
            nc.gpsimd.dma_start(out=xet[2][:], in_=xe[2])
            nc.gpsimd.dma_start(out=xet[3][:], in_=xe[3])

            # Panel remainders (needed from the second matmul group on),
            # then the remaining activation chunks in consumption order.
            nc.sync.dma_start(out=w1t0[:, P:H], in_=w1p[0][:, P:H])
            nc.scalar.dma_start(out=w3t0[:, P:H], in_=w3p[0][:, P:H])
            nc.sync.dma_start(out=w1t1[:, P:H], in_=w1p[1][:, P:H])
            nc.scalar.dma_start(out=w3t1[:, P:H], in_=w3p[1][:, P:H])
            for hk in range(4, HK):
                e = nc.sync if hk % 2 == 0 else nc.scalar
                e.dma_start(out=xet[hk][:], in_=xe[hk])

            gt = [gpool.tile([P, C], bf, tag=f"g{fp}", name=f"g{fp}")
                  for fp in range(FP)]

            # Phase A: h1T/h3T = w1/w3 @ xeT per 128-row chunk of F,
            # fused SwiGLU into gT (bf16).
            with tc.tile_pool(name="psA", bufs=4, space="PSUM") as psA:
                # Tagged ps1 so it occupies one rotation slot of that tag;
                # the first real ps1 chain gets the next slot and starts
                # without waiting on the warmup matmuls.
                wps = psA.tile([P, 512], f32, tag="ps1", name="wps")
                for i in range(NWARM):
                    nc.tensor.matmul(wps[:], warm[:, 0:P], warm[:],
                                     start=True, stop=True)
                # fp0 + fp1 in hk-LOCKSTEP: 8 matmuls consume each xe[hk]
                # chunk as it lands (~1.7us of PE per 256KB chunk), so the
                # startup DMA stream keeps ahead of the PE with margin.
                pss = {}
                for fp in (0, 1):
                    for mat in (1, 3):
                        for ci in range(len(cn_chunks)):
                            pss[(fp, mat, ci)] = psA.tile(
                                [P, 512], f32, tag=f"ps{mat}",
                                name=f"ps{mat}_f{fp}_c{ci}",
                            )
                wts = {(0, 1): w1t0, (0, 3): w3t0, (1, 1): w1t1, (1, 3): w3t1}
                for hk in range(HK):
                    for fp in (0, 1):
                        for mat in (1, 3):
                            wt = wts[(fp, mat)]
                            for ci, (coff, csz) in enumerate(cn_chunks):
                                nc.tensor.matmul(
                                    pss[(fp, mat, ci)][:, :csz],
                                    wt[:, hk * P:(hk + 1) * P],
                                    xet[hk][:, coff:coff + csz],
                                    start=(hk == 0), stop=(hk == HK - 1),
                                )
                for fp in (0, 1):
                    for ci, (coff, csz) in enumerate(cn_chunks):
                        sil = evac.tile([P, 512], f32, tag="sil",
                                        name=f"sil_f{fp}_{ci}")
                        nc.scalar.activation(
                            sil[:, :csz], pss[(fp, 1, ci)][:, :csz], silu)
                        nc.vector.tensor_mul(
                            gt[fp][:, coff:coff + csz], sil[:, :csz],
                            pss[(fp, 3, ci)][:, :csz],
                        )
                for fp in range(2, FP):
                    w1t = wload.tile([P, H], bf, tag="w1")
                    nc.sync.dma_start(out=w1t[:], in_=w1p[fp])
                    w3t = wload.tile([P, H], bf, tag="w3")
                    nc.sync.dma_start(out=w3t[:], in_=w3p[fp])
                    for (coff, csz) in cn_chunks:
                        ps1 = psA.tile([P, 512], f32, tag="ps1")
                        ps3 = psA.tile([P, 512], f32, tag="ps3")
                        for hk in range(HK):
                            nc.tensor.matmul(
                                ps1[:, :csz],
                                w1t[:, hk * P:(hk + 1) * P],
                                xet[hk][:, coff:coff + csz],
                                start=(hk == 0), stop=(hk == HK - 1),
                            )
                        for hk in range(HK):
                            nc.tensor.matmul(
                                ps3[:, :csz],
                                w3t[:, hk * P:(hk + 1) * P],
                                xet[hk][:, coff:coff + csz],
                                start=(hk == 0), stop=(hk == HK - 1),
                            )
                        sil = evac.tile([P, 512], f32, tag="sil")
                        nc.scalar.activation(sil[:, :csz], ps1[:, :csz], silu)
                        nc.vector.tensor_mul(
                            gt[fp][:, coff:coff + csz], sil[:, :csz], ps3[:, :csz]
                        )

            # w2 / routing-weight loads are only needed in phase B; emit
            # them after phase A so the DMA queues serve phase A first.
            cvt = []
            for ck in range(CK):
                t = persist.tile([P, 1], f32, tag=f"cv{ck}", name=f"cv{ck}")
                nc.sync.dma_start(out=t[:], in_=cv[ck * P:(ck + 1) * P, :])
                cvt.append(t)
            w2t = []
            for fp in range(FP):
                t = persist.tile([P, H], bf, tag=f"w2_{fp}", name=f"w2_{fp}")
                nc.sync.dma_start(out=t[:], in_=w2p[fp])
                w2t.append(t)

            # Phase B: outT chunk [128 tokens, 1024] = sum_f gT.T @ w2T,
            # scaled by the per-token routing weight on eviction.  The two
            # 512-column halves run as back-to-back full accumulation
            # chains; each half is evicted (bf16) and DMA'd as soon as its
            # chain completes — pb0 on Scalar+Sync, pb1 on Vector+Scalar —
            # so the end-of-kernel drain is one half-eviction deep.
            with tc.tile_pool(name="psB", bufs=4, space="PSUM") as psB:
                for ck in range(CK):
                    pb0 = psB.tile([P, 512], f32, tag="pb0")
                    pb1 = psB.tile([P, 512], f32, tag="pb1")
                    for fp in range(FP):
                        nc.tensor.matmul(pb0[:], gt[fp][:, ck * P:(ck + 1) * P],
                                         w2t[fp][:, 0:512],
                                         start=(fp == 0), stop=(fp == FP - 1))
                    o0 = ost.tile([P, 512], bf, tag="o0")
                    nc.scalar.activation(o0[:], pb0[:], copy,
                                         scale=cvt[ck][:])
                    nc.sync.dma_start(out=out[ck * P:(ck + 1) * P, 0:512],
                                      in_=o0[:])
                    for fp in range(FP):
                        nc.tensor.matmul(pb1[:], gt[fp][:, ck * P:(ck + 1) * P],
                                         w2t[fp][:, 512:1024],
                                         start=(fp == 0), stop=(fp == FP - 1))
                    o1 = ost.tile([P, 512], bf, tag="o1")
                    if ck == CK - 1:
                        # Final chunk: evict the two 256-col quarters on
                        # different engines with separate DMAs to minimize
                        # the end-of-kernel drain depth.
                        nc.scalar.activation(o1[:, 0:256], pb1[:, 0:256],
                                             copy, scale=cvt[ck][:])
                        nc.scalar.dma_start(
                            out=out[ck * P:(ck + 1) * P, 512:768],
                            in_=o1[:, 0:256])
                        nc.vector.tensor_scalar_mul(o1[:, 256:512],
                                                    pb1[:, 256:512],
                                                    cvt[ck][:])
                        nc.sync.dma_start(
                            out=out[ck * P:(ck + 1) * P, 768:1024],
                            in_=o1[:, 256:512])
                    else:
                        nc.vector.tensor_scalar_mul(o1[:], pb1[:], cvt[ck][:])
                        nc.scalar.dma_start(
                            out=out[ck * P:(ck + 1) * P, 512:1024],
                            in_=o1[:])

    nc.compile()
    return nc


def _silu(v):
    return v / (1.0 + np.exp(-v))


def kernel(hidden_states, gate_w, w1, w2, w3, _trace=False):
    global LAST_EXEC_TIME_NS
    _ensure_axon_hooks_stub()
    from concourse.bass_utils import run_bass_kernel_spmd

    x = np.asarray(hidden_states, dtype=np.float32).reshape(-1, H)
    gate_w = np.asarray(gate_w, dtype=np.float32)
    w1 = np.asarray(w1, dtype=np.float32)
    w2 = np.asarray(w2, dtype=np.float32)
    w3 = np.asarray(w3, dtype=np.float32)
    T = x.shape[0]

    # Router (f32, same math as the module): softmax over experts, top-2,
    # renormalized weights.
    logits = x @ gate_w.T
    p = np.exp(logits - logits.max(-1, keepdims=True))
    p /= p.sum(-1, keepdims=True)
    sel = np.argpartition(-p, TOP_K - 1, axis=-1)[:, :TOP_K]
    rw = np.take_along_axis(p, sel, axis=-1)
    rw = rw / rw.sum(-1, keepdims=True)

    idx_e, cv_e = [], []
    for e in range(E):
        hit = sel == e                      # [T, K]
        idx = np.nonzero(hit.any(axis=1))[0]
        w = np.where(hit[idx, 0], rw[idx, 0], rw[idx, 1])
        idx_e.append(idx)
        cv_e.append(w.astype(np.float32))

    if "nc" not in _BUILD_CACHE:
        _BUILD_CACHE["nc"] = _build()
    nc = _BUILD_CACHE["nc"]

    x_bf = x.astype(BF16)
    in_maps = []
    for e in range(E):
        idx = idx_e[e][:C]
        n = len(idx)
        xeT = np.zeros((H, C), dtype=BF16)
        xeT[:, :n] = x_bf[idx].T
        cvp = np.zeros((C, 1), dtype=np.float32)
        cvp[:n, 0] = cv_e[e][:n]
        w1pk = np.ascontiguousarray(
            w1[e].astype(BF16).reshape(FP, P, HK, P).transpose(0, 3, 2, 1)
        ).reshape(FP, P, H)
        w3pk = np.ascontiguousarray(
            w3[e].astype(BF16).reshape(FP, P, HK, P).transpose(0, 3, 2, 1)
        ).reshape(FP, P, H)
        w2pk = np.ascontiguousarray(w2[e].T.astype(BF16)).reshape(FP, P, H)
        in_maps.append({
            "xe": np.ascontiguousarray(xeT.reshape(HK, P, C)),
            "w1p": w1pk,
            "w3p": w3pk,
            "w2p": w2pk,
            "cv": cvp,
        })

    res = run_bass_kernel_spmd(
        nc, in_maps, core_ids=list(range(N_CORES)), trace=_trace
    )
    LAST_EXEC_TIME_NS = res.exec_time_ns

    out = np.zeros((T, H), dtype=np.float32)
    for e in range(E):
        idx = idx_e[e][:C]
        n = len(idx)
        if n:
            dev = np.asarray(res.results[e]["out"], dtype=np.float32)
            out[idx] += dev.reshape(C, H)[:n]
        # Capacity overflow (tokens beyond C for this expert): exact
        # host-side fp32 patch.  ~1.3% of pairs at balanced routing.
        ov = idx_e[e][C:]
        if len(ov):
            X = x[ov]
            h1 = X @ w1[e].T
            h3 = X @ w3[e].T
            g = _silu(h1) * h3 * cv_e[e][C:, None]
            out[ov] += g @ w2[e].T
    return out.reshape(B, S, H)


# revision 17
# speedup vs baseline: 1.1967x; 1.0053x over previous
"""Mixtral-style MoE (top-2 of 8 experts) on 8 TRN2 NeuronCores.

Strategy (expert-parallel, matching TENSOR_EXPERT_PARALLEL):
  - Host: router (logits -> softmax -> top-2 -> normalized weights), then
    shard: core e receives up to C=1024 tokens routed to expert e
    (gathered and pre-transposed to [H, C]) plus expert e's w1/w3/w2
    (bf16, pre-packed into PE-friendly [128 x free] tiles).  C=1024 is
    chosen so every core does identical, full-partition work (8 chunks
    of 128 tokens); the few overflow pairs beyond 1024 per expert
    (~1.3% of the 8192 token-expert pairs at balanced routing) are
    computed exactly on the host in fp32 and added into the output.
  - Device (SPMD, identical program on 8 cores): h1T = w1 @ xeT,
    h3T = w3 @ xeT, gT = silu(h1T) * h3T (bf16), outT = gT.T @ w2T,
    scaled per-token by the routing weight.  Pure GEMM pipeline; all
    DMAs are fully linear.
  - Host: scatter-add each core's [count_e, H] contribution into the
    [T, H] output (each token appears in exactly TOP_K=2 expert lists).

Compute is done in bf16 (fp32 accumulation in PSUM), which keeps the
TensorEngine at its 78.6 TF/s peak; sparse routing means each core does
C = 1024 token-columns instead of all 4096 (4x fewer FLOPs than dense).
"""

import numpy as np
import ml_dtypes

B, S, H, F, E, TOP_K = 2, 2048, 1024, 3584, 8, 2
N_CORES = 8
P = 128
HK = H // P   # 8 contraction chunks for up-proj
FP = F // P   # 28 partition chunks of the FFN dim
C = 1024      # per-core token capacity (8 full 128-token chunks)
CK = C // P
NWARM = 11    # HAM warmup matmuls (PE clock-gate ramp)

BF16 = ml_dtypes.bfloat16

_BUILD_CACHE = {}
LAST_EXEC_TIME_NS = None


def _ensure_axon_hooks_stub():
    """bass_utils imports antenv.axon_hooks when BASS_TRACE is set; the
    agent image lacks it.  Register a None-hook stub so a stray
    BASS_TRACE env var degrades to an untraced run instead of crashing.
    """
    import sys, types

    try:
        import antenv.axon_hooks  # noqa: F401
        return
    except ImportError:
        pass
    mod = types.ModuleType("antenv.axon_hooks")
    mod._hook = None
    mod.set_axon_ntff_profile_hook = lambda h: setattr(mod, "_hook", h)
    mod.get_axon_ntff_profile_hook = lambda: mod._hook
    sys.modules["antenv.axon_hooks"] = mod
    try:
        import antenv

        antenv.axon_hooks = mod
    except ImportError:
        pass


def _build():
    """Build + compile the SPMD Bass program (token capacity C=1024)."""
    import concourse.bacc as bacc
    import concourse.mybir as mybir
    from concourse.tile import TileContext

    bf = mybir.dt.bfloat16
    f32 = mybir.dt.float32

    nc = bacc.Bacc("TRN2", target_bir_lowering=False, debug=False,
                   num_devices=N_CORES)
    xe = nc.dram_tensor("xe", [HK, P, C], bf, kind="ExternalInput")
    w1p = nc.dram_tensor("w1p", [FP, P, H], bf, kind="ExternalInput")
    w3p = nc.dram_tensor("w3p", [FP, P, H], bf, kind="ExternalInput")
    w2p = nc.dram_tensor("w2p", [FP, P, H], bf, kind="ExternalInput")
    cv = nc.dram_tensor("cv", [C, 1], f32, kind="ExternalInput")
    out = nc.dram_tensor("out", [C, H], bf, kind="ExternalOutput")

    cn_chunks = [(0, 512), (512, 512)]
    silu = mybir.ActivationFunctionType.Silu
    copy = mybir.ActivationFunctionType.Copy

    with TileContext(nc) as tc:
        with (
            tc.tile_pool(name="persist", bufs=1) as persist,
            tc.tile_pool(name="wload", bufs=3) as wload,
            tc.tile_pool(name="gpool", bufs=1) as gpool,
            tc.tile_pool(name="evac", bufs=4) as evac,
            tc.tile_pool(name="ost", bufs=3) as ost,
        ):
            # Startup-critical DMA order: fp0 panels, first activation
            # chunks, fp1 panels, rest of the activations.  fp0's hk-outer
            # matmul chain consumes xe[hk] at ~1us/chunk, so fp1's panels
            # slot in without delaying it.  DIRECT2D issue costs ~0.6us
            # each on a sequencer, so the critical set is issued first,
            # split across the two HWDGE engines (Sync + Scalar).
            # The fp0/fp1 panels are split into 512-column halves: the
            # first halves (hk0-3, 1KB-per-partition descriptors at DMA
            # line rate) are the startup-critical set together with
            # xe[0]; the second halves ride behind and land well before
            # the lockstep reaches hk4.
            w1t0 = wload.tile([P, H], bf, tag="w1")
            w3t0 = wload.tile([P, H], bf, tag="w3")
            w1t1 = wload.tile([P, H], bf, tag="w1")
            w3t1 = wload.tile([P, H], bf, tag="w3")
            # Critical chunks: hk0-1 columns (64KB, 512B-per-partition
            # descriptors — the smallest line-rate transfer), so the
            # first matmul groups unblock as early as possible.
            HQ = 2 * P
            nc.sync.dma_start(out=w1t0[:, 0:HQ], in_=w1p[0][:, 0:HQ])
            nc.scalar.dma_start(out=w3t0[:, 0:HQ], in_=w3p[0][:, 0:HQ])
            nc.sync.dma_start(out=w1t1[:, 0:HQ], in_=w1p[1][:, 0:HQ])
            nc.scalar.dma_start(out=w3t1[:, 0:HQ], in_=w3p[1][:, 0:HQ])

            xet = [persist.tile([P, C], bf, tag=f"xe{hk}", name=f"xe{hk}")
                   for hk in range(HK)]
            # xe[0] next: split across both engines so its completion
            # semaphores fire ~1us sooner.
            nc.sync.dma_start(out=xet[0][:, 0:512], in_=xe[0][:, 0:512])
            nc.scalar.dma_start(out=xet[0][:, 512:C], in_=xe[0][:, 512:C])

            # GpSimd's SWDGE is a THIRD parallel DMA-issue stream for
            # xe[1] — needed while the two HWDGE queues still drain the
            # panel remainders.  The HAM warmup operand memset runs on
            # the otherwise-idle Vector engine so GpSimd issues this
            # immediately.
            warm = persist.tile([P, 512], bf, tag="warm", name="warm")
            nc.vector.memset(warm[:], 0.0)
            nc.gpsimd.dma_start(out=xet[1][:], in_=xe[1])

            # Panel remainders (needed from hk2 on), then the remaining
            # activation chunks in consumption order.
            nc.sync.dma_start(out=w1t0[:, HQ:H], in_=w1p[0][:, HQ:H])
            nc.scalar.dma_start(out=w3t0[:, HQ:H], in_=w3p[0][:, HQ:H])
            nc.sync.dma_start(out=w1t1[:, HQ:H], in_=w1p[1][:, HQ:H])
            nc.scalar.dma_start(out=w3t1[:, HQ:H], in_=w3p[1][:, HQ:H])
            for hk in range(2, HK):
                e = nc.sync if hk % 2 == 0 else nc.scalar
                e.dma_start(out=xet[hk][:], in_=xe[hk])

            gt = [gpool.tile([P, C], bf, tag=f"g{fp}", name=f"g{fp}")
                  for fp in range(FP)]

            # Phase A: h1T/h3T = w1/w3 @ xeT per 128-row chunk of F,
            # fused SwiGLU into gT (bf16).
            with tc.tile_pool(name="psA", bufs=4, space="PSUM") as psA:
                # Tagged ps1 so it occupies one rotation slot of that tag;
                # the first real ps1 chain gets the next slot and starts
                # without waiting on the warmup matmuls.
                wps = psA.tile([P, 512], f32, tag="ps1", name="wps")
                for i in range(NWARM):
                    nc.tensor.matmul(wps[:], warm[:, 0:P], warm[:],
                                     start=True, stop=True)
                # fp0 + fp1 in hk-LOCKSTEP: 8 matmuls consume each xe[hk]
                # chunk as it lands (~1.7us of PE per 256KB chunk), so the
                # startup DMA stream keeps ahead of the PE with margin.
                pss = {}
                for fp in (0, 1):
                    for mat in (1, 3):
                        for ci in range(len(cn_chunks)):
                            pss[(fp, mat, ci)] = psA.tile(
                                [P, 512], f32, tag=f"ps{mat}",
                                name=f"ps{mat}_f{fp}_c{ci}",
                            )
                wts = {(0, 1): w1t0, (0, 3): w3t0, (1, 1): w1t1, (1, 3): w3t1}
                for hk in range(HK):
                    for fp in (0, 1):
                        for mat in (1, 3):
                            wt = wts[(fp, mat)]
                            for ci, (coff, csz) in enumerate(cn_chunks):
                                nc.tensor.matmul(
                                    pss[(fp, mat, ci)][:, :csz],
                                    wt[:, hk * P:(hk + 1) * P],
                                    xet[hk][:, coff:coff + csz],
                                    start=(hk == 0), stop=(hk == HK - 1),
                                )
                for fp in (0, 1):
                    for ci, (coff, csz) in enumerate(cn_chunks):
                        sil = evac.tile([P, 512], f32, tag="sil",
                                        name=f"sil_f{fp}_{ci}")
                        nc.scalar.activation(
                            sil[:, :csz], pss[(fp, 1, ci)][:, :csz], silu)
                        nc.vector.tensor_mul(
                            gt[fp][:, coff:coff + csz], sil[:, :csz],
                            pss[(fp, 3, ci)][:, :csz],
                        )
                for fp in range(2, FP):
                    w1t = wload.tile([P, H], bf, tag="w1")
                    nc.sync.dma_start(out=w1t[:], in_=w1p[fp])
                    w3t = wload.tile([P, H], bf, tag="w3")
                    nc.sync.dma_start(out=w3t[:], in_=w3p[fp])
                    # Reversed chunk order on the last fp: its ci0 PSUM
                    # tiles then land on the rotation slots phase B's
                    # first chains reuse, and are evicted ~2us before
                    # phase A ends — no A->B PSUM-bank wait.
                    chunks = (list(reversed(cn_chunks)) if fp == FP - 1
                              else cn_chunks)
                    for ci, (coff, csz) in enumerate(chunks):
                        last_chunk = fp == FP - 1 and ci == len(chunks) - 1
                        ps1 = psA.tile([P, 512], f32, tag="ps1")
                        ps3 = psA.tile([P, 512], f32, tag="ps3")
                        for hk in range(HK):
                            nc.tensor.matmul(
                                ps1[:, :csz],
                                w1t[:, hk * P:(hk + 1) * P],
                                xet[hk][:, coff:coff + csz],
                                start=(hk == 0), stop=(hk == HK - 1),
                            )
                        sil = evac.tile([P, 512], f32, tag="sil")
                        if last_chunk:
                            # Final chunk of phase A: split the ps3
                            # accumulation into 256-col halves so the last
                            # gt eviction (which gates phase B's first
                            # chain) trails the last matmul by only ~0.35us.
                            for (qo, qs) in ((0, 256), (256, csz - 256)):
                                for hk in range(HK):
                                    nc.tensor.matmul(
                                        ps3[:, qo:qo + qs],
                                        w3t[:, hk * P:(hk + 1) * P],
                                        xet[hk][:, coff + qo:coff + qo + qs],
                                        start=(hk == 0), stop=(hk == HK - 1),
                                    )
                                nc.scalar.activation(
                                    sil[:, qo:qo + qs], ps1[:, qo:qo + qs],
                                    silu)
                                nc.vector.tensor_mul(
                                    gt[fp][:, coff + qo:coff + qo + qs],
                                    sil[:, qo:qo + qs], ps3[:, qo:qo + qs],
                                )
                            continue
                        for hk in range(HK):
                            nc.tensor.matmul(
                                ps3[:, :csz],
                                w3t[:, hk * P:(hk + 1) * P],
                                xet[hk][:, coff:coff + csz],
                                start=(hk == 0), stop=(hk == HK - 1),
                            )
                        nc.scalar.activation(sil[:, :csz], ps1[:, :csz], silu)
                        nc.vector.tensor_mul(
                            gt[fp][:, coff:coff + csz], sil[:, :csz], ps3[:, :csz]
                        )

            # w2 / routing-weight loads are only needed in phase B; emit
            # them after phase A so the DMA queues serve phase A first.
            cvt = []
            for ck in range(CK):
                t = persist.tile([P, 1], f32, tag=f"cv{ck}", name=f"cv{ck}")
                nc.sync.dma_start(out=t[:], in_=cv[ck * P:(ck + 1) * P, :])
                cvt.append(t)
            w2t = []
            for fp in range(FP):
                t = persist.tile([P, H], bf, tag=f"w2_{fp}", name=f"w2_{fp}")
                nc.sync.dma_start(out=t[:], in_=w2p[fp])
                w2t.append(t)

            # Phase B: outT chunk [128 tokens, 1024] = sum_f gT.T @ w2T,
            # scaled by the per-token routing weight on eviction.  The two
            # 512-column halves run as back-to-back full accumulation
            # chains; each half is evicted (bf16) and DMA'd as soon as its
            # chain completes — pb0 on Scalar+Sync, pb1 on Vector+Scalar —
            # so the end-of-kernel drain is one half-eviction deep.
            with tc.tile_pool(name="psB", bufs=4, space="PSUM") as psB:
                for ck in range(CK):
                    pb0 = psB.tile([P, 512], f32, tag="pb0")
                    pb1 = psB.tile([P, 512], f32, tag="pb1")
                    for fp in range(FP):
                        nc.tensor.matmul(pb0[:], gt[fp][:, ck * P:(ck + 1) * P],
                                         w2t[fp][:, 0:512],
                                         start=(fp == 0), stop=(fp == FP - 1))
                    o0 = ost.tile([P, 512], bf, tag="o0")
                    nc.scalar.activation(o0[:], pb0[:], copy,
                                         scale=cvt[ck][:])
                    nc.sync.dma_start(out=out[ck * P:(ck + 1) * P, 0:512],
                                      in_=o0[:])
                    for fp in range(FP):
                        nc.tensor.matmul(pb1[:], gt[fp][:, ck * P:(ck + 1) * P],
                                         w2t[fp][:, 512:1024],
                                         start=(fp == 0), stop=(fp == FP - 1))
                    o1 = ost.tile([P, 512], bf, tag="o1")
                    if ck == CK - 1:
                        # Final chunk: evict the two 256-col quarters on
                        # different engines with separate DMAs to minimize
                        # the end-of-kernel drain depth.
                        nc.scalar.activation(o1[:, 0:256], pb1[:, 0:256],
                                             copy, scale=cvt[ck][:])
                        nc.scalar.dma_start(
                            out=out[ck * P:(ck + 1) * P, 512:768],
                            in_=o1[:, 0:256])
                        nc.vector.tensor_scalar_mul(o1[:, 256:512],
                                                    pb1[:, 256:512],
                                                    cvt[ck][:])
                        nc.sync.dma_start(
                            out=out[ck * P:(ck + 1) * P, 768:1024],
                            in_=o1[:, 256:512])
                    else:
                        nc.vector.tensor_scalar_mul(o1[:], pb1[:], cvt[ck][:])
                        nc.scalar.dma_start(
                            out=out[ck * P:(ck + 1) * P, 512:1024],
                            in_=o1[:])

    nc.compile()
    return nc


def _silu(v):
    return v / (1.0 + np.exp(-v))


def kernel(hidden_states, gate_w, w1, w2, w3, _trace=False):
    global LAST_EXEC_TIME_NS
    _ensure_axon_hooks_stub()
    from concourse.bass_utils import run_bass_kernel_spmd

    x = np.asarray(hidden_states, dtype=np.float32).reshape(-1, H)
    gate_w = np.asarray(gate_w, dtype=np.float32)
    w1 = np.asarray(w1, dtype=np.float32)
    w2 = np.asarray(w2, dtype=np.float32)
    w3 = np.asarray(w3, dtype=np.float32)
    T = x.shape[0]

    # Router (f32, same math as the module): softmax over experts, top-2,
    # renormalized weights.
    logits = x @ gate_w.T
    p = np.exp(logits - logits.max(-1, keepdims=True))
    p /= p.sum(-1, keepdims=True)
    sel = np.argpartition(-p, TOP_K - 1, axis=-1)[:, :TOP_K]
    rw = np.take_along_axis(p, sel, axis=-1)
    rw = rw / rw.sum(-1, keepdims=True)

    idx_e, cv_e = [], []
    for e in range(E):
        hit = sel == e                      # [T, K]
        idx = np.nonzero(hit.any(axis=1))[0]
        w = np.where(hit[idx, 0], rw[idx, 0], rw[idx, 1])
        idx_e.append(idx)
        cv_e.append(w.astype(np.float32))

    if "nc" not in _BUILD_CACHE:
        _BUILD_CACHE["nc"] = _build()
    nc = _BUILD_CACHE["nc"]

    x_bf = x.astype(BF16)
    in_maps = []
    for e in range(E):
        idx = idx_e[e][:C]
        n = len(idx)
        xeT = np.zeros((H, C), dtype=BF16)
        xeT[:, :n] = x_bf[idx].T
        cvp = np.zeros((C, 1), dtype=np.float32)
        cvp[:n, 0] = cv_e[e][:n]
        w1pk = np.ascontiguousarray(
            w1[e].astype(BF16).reshape(FP, P, HK, P).transpose(0, 3, 2, 1)
        ).reshape(FP, P, H)
        w3pk = np.ascontiguousarray(
            w3[e].astype(BF16).reshape(FP, P, HK, P).transpose(0, 3, 2, 1)
        ).reshape(FP, P, H)
        w2pk = np.ascontiguousarray(w2[e].T.astype(BF16)).reshape(FP, P, H)
        in_maps.append({
            "xe": np.ascontiguousarray(xeT.reshape(HK, P, C)),
            "w1p": w1pk,
            "w3p": w3pk,
            "w2p": w2pk,
            "cv": cvp,
        })

    res = run_bass_kernel_spmd(
        nc, in_maps, core_ids=list(range(N_CORES)), trace=_trace
    )
    LAST_EXEC_TIME_NS = res.exec_time_ns

    out = np.zeros((T, H), dtype=np.float32)
    for e in range(E):
        idx = idx_e[e][:C]
        n = len(idx)
        if n:
            dev = np.asarray(res.results[e]["out"], dtype=np.float32)
            out[idx] += dev.reshape(C, H)[:n]
        # Capacity overflow (tokens beyond C for this expert): exact
        # host-side fp32 patch.  ~1.3% of pairs at balanced routing.
        ov = idx_e[e][C:]
        if len(ov):
            X = x[ov]
            h1 = X @ w1[e].T
            h3 = X @ w3[e].T
            g = _silu(h1) * h3 * cv_e[e][C:, None]
            out[ov] += g @ w2[e].T
    return out.reshape(B, S, H)
